# revision 10
# baseline (speedup 1.0000x reference)
"""PointCloudAE forward pass on 8 Trainium2 NeuronCores (Bass/Tile).

Strategy: data-parallel over batch B=16 -> 2 items per core. Each core runs
encoder (1x1 convs + global max pool), decoder MLP + layernorm, and the
chamfer distance terms for its 2 items; the scalar loss partial is summed on
host across cores (the only cross-core reduction).

Key kernel-level choices:
- All heavy matmuls in float32r (full PE rate, ~1.6e-4 rel err) except fc3
  (bf16 weights, 12 MiB instead of 24 MiB in SBUF).
- x0's upper half (broadcast global feature) is rank-1: conv3's second
  K-tile collapses to a per-channel constant c3 = W3b @ gfeat, halving
  conv3 matmul work.
- Chamfer distance matrix is produced directly by the PE via augmented
  K=4 matmuls: [px,py,pz,1] . [-2qx,-2qy,-2qz,q2] = q2 - 2 p.q. The
  p2[n]/q2[m] terms fall out of the min and are summed separately
  (mean(d1) = mean(min-term) + mean(p2)), so no layout pairing is needed.
- Min/max reductions over PSUM: ACT evacuates half of each strip to SBUF,
  DVE does tensor_tensor-min against the other PSUM half, then reduce_min.
"""
import numpy as np
import ml_dtypes
from contextlib import ExitStack

import concourse.mybir as mybir
import concourse.tile as tile
from concourse import bacc
from concourse.bass_utils import run_bass_kernel_spmd
from concourse.bass import ts

F32 = mybir.dt.float32
F32R = mybir.dt.float32r
BF16 = mybir.dt.bfloat16
AL = mybir.AluOpType
AF = mybir.ActivationFunctionType
AX = mybir.AxisListType

B, N, NPTS = 16, 4096, 2048
B_L = 2                      # items per core
NCORES = 8
EMB = 256
H6 = 3 * NPTS                # 6144
LN_EPS = 1e-5
NEG_INF = -3.0e38
POS_INF = 3.0e38

_CACHE = {}


# --------------------------------------------------------------------------
# program emission
# --------------------------------------------------------------------------

def _emit(nc, tc, ctx, tensors):
    (pc_raw, w1T, b1, w2T, b2, w3aT, w3bT, b3c, fencT, fbe, f1T, fb1,
     f2T, fb2, f3T, fb3, g_ln, bt_ln, ones4096, wloss,
     x0_out, emb_out, pc_out, loss_out) = tensors

    pw = ctx.enter_context(tc.tile_pool(name="weights", bufs=1))
    px = ctx.enter_context(tc.tile_pool(name="work", bufs=1))
    pr = ctx.enter_context(tc.tile_pool(name="red", bufs=3))
    pp = ctx.enter_context(tc.tile_pool(name="ps", bufs=2, space="PSUM"))

    def psum(p_, f_):
        return pp.tile([p_, f_], F32, tag="ps", name="ps")

    # ---- load persistent weights ------------------------------------------
    w1T_sb = pw.tile([3, 64], F32R)
    nc.sync.dma_start(w1T_sb[:], w1T[:])
    b1_sb = pw.tile([64, 1], F32)
    nc.sync.dma_start(b1_sb[:], b1[:])
    w2T_sb = pw.tile([64, 128], F32R)
    nc.sync.dma_start(w2T_sb[:], w2T[:])
    b2_sb = pw.tile([128, 1], F32)
    nc.sync.dma_start(b2_sb[:], b2[:])
    w3aT_sb = pw.tile([128, 1024], F32R)
    nc.sync.dma_start(w3aT_sb[:], w3aT[:])
    w3bT_sb = pw.tile([128, 1024], F32R)
    nc.sync.dma_start(w3bT_sb[:], w3bT[:])
    b3c_sb = pw.tile([128, 8], F32)
    nc.sync.dma_start(b3c_sb[:], b3c[:])
    fencT_sb = [pw.tile([128, 256], F32R, tag=f"fencT{t}", name=f"fencT{t}") for t in range(8)]
    for t in range(8):
        nc.sync.dma_start(fencT_sb[t][:], fencT[ts(t, 128), :])
    fbe_sb = pw.tile([128, 2], F32)
    nc.sync.dma_start(fbe_sb[:], fbe[:])
    f1T_sb = [pw.tile([128, 512], F32R, tag=f"f1T{t}", name=f"f1T{t}") for t in range(2)]
    for t in range(2):
        nc.sync.dma_start(f1T_sb[t][:], f1T[ts(t, 128), :])
    fb1_sb = pw.tile([128, 4], F32)
    nc.sync.dma_start(fb1_sb[:], fb1[:])
    f2T_sb = [pw.tile([128, 1024], F32R, tag=f"f2T{t}", name=f"f2T{t}") for t in range(4)]
    for t in range(4):
        nc.sync.dma_start(f2T_sb[t][:], f2T[ts(t, 128), :])
    fb2_sb = pw.tile([128, 8], F32)
    nc.sync.dma_start(fb2_sb[:], fb2[:])
    f3T_sb = [pw.tile([128, H6], BF16, tag=f"f3T{t}", name=f"f3T{t}") for t in range(8)]
    for t in range(8):
        nc.sync.dma_start(f3T_sb[t][:], f3T[ts(t, 128), :])
    fb3_sb = pw.tile([128, 48], F32)
    nc.sync.dma_start(fb3_sb[:], fb3[:])
    g_sb = pw.tile([128, 48], F32)
    nc.sync.dma_start(g_sb[:], g_ln[:])
    bt_sb = pw.tile([128, 48], F32)
    nc.sync.dma_start(bt_sb[:], bt_ln[:])
    ones128_sb = pw.tile([128, 1], F32)
    nc.vector.memset(ones128_sb[:], 1.0)
    wloss_sb = pw.tile([1, 4], F32)
    nc.sync.dma_start(wloss_sb[:], wloss[:])

    # per-item state shared with decoder/chamfer
    xmax_all = pw.tile([128, 8, 2], F32R)    # encoder output (channels, ktile, item)
    loss_acc = pw.tile([1, 1], F32)
    nc.vector.memset(loss_acc[:], 0.0)
    stats4 = [pw.tile([128, 4], F32, tag=f"stats{b}", name=f"stats{b}") for b in range(B_L)]
    for b in range(B_L):
        nc.vector.memset(stats4[b][:], 0.0)
    # stats cols: 0=e1sum 1=p2sum 2=e2sum 3=q2sum (each [128,1] partials)

    # scratch DRAM for partition-flatten bounces (per item)
    scrA = [nc.dram_tensor(f"scrA{b}", [4, N], F32R) for b in range(B_L)]
    scrB = [nc.dram_tensor(f"scrB{b}", [4, N], F32R) for b in range(B_L)]
    scrQA = [nc.dram_tensor(f"scrQA{b}", [4, NPTS], F32R) for b in range(B_L)]
    scrQB = [nc.dram_tensor(f"scrQB{b}", [4, NPTS], F32R) for b in range(B_L)]

    def flat_out(dram_row, sb_tile, pcount):
        """DMA [128, F] sbuf tile -> one dram row (partition-major flatten)."""
        nc.sync.dma_start(
            dram_row.rearrange("o (p f) -> (o p) f", p=128), sb_tile[:])

    # One P-slot [4, N] and one Q-slot [4, NPTS], reloaded from DRAM scratch
    # at each use phase (enc-b: P_A[b]; cham-b d1: P_A[b]+Q_B[b]; d2: P_B[b]+Q_A[b]).
    def p_slot():
        return px.tile([4, N], F32R, tag="Pslot", name="Pslot")

    def q_slot():
        return px.tile([4, NPTS], F32R, tag="Qslot", name="Qslot")

    h3_all = pw.tile([128, 48, 2], F32)

    # ======================= per-item encoder ==============================
    for b in range(B_L):
        raw = px.tile([128, 96], F32, tag="raw")
        nc.sync.dma_start(raw[:], pc_raw[b, :, :])
        rv = raw[:].rearrange("p (a c) -> p a c", c=3)

        # coordinate planes + squared norm (point n = 32p + j)
        plane = [px.tile([128, 32], F32R, tag=f"plane{c}", name=f"plane{c}") for c in range(3)]
        nplane = [px.tile([128, 32], F32R, tag=f"nplane{c}", name=f"nplane{c}") for c in range(3)]
        for c in range(3):
            nc.vector.tensor_scalar(plane[c][:], rv[:, :, c], 0.0, None, AL.add)
            nc.vector.tensor_scalar(nplane[c][:], rv[:, :, c], -2.0, None, AL.mult)
        sq = px.tile([128, 96], F32, tag="sq")
        nc.scalar.activation(sq[:], raw[:], AF.Square)
        sv = sq[:].rearrange("p (a c) -> p a c", c=3)
        t2 = px.tile([128, 32], F32, tag="t2")
        nc.vector.tensor_add(t2[:], sv[:, :, 0], sv[:, :, 1])
        nc.vector.tensor_add(t2[:], t2[:], sv[:, :, 2])
        p2r = px.tile([128, 32], F32R, tag="p2r")
        nc.vector.tensor_scalar(p2r[:], t2[:], 0.0, None, AL.add)
        # p2 total for the loss
        nc.vector.reduce_sum(stats4[b][:, 1:2], t2[:], axis=AX.X)

        # bounce planes to DRAM, load P_A/P_B rows
        for c in range(3):
            flat_out(scrA[b][c : c + 1, :], plane[c], 128)
            flat_out(scrB[b][c : c + 1, :], nplane[c], 128)
        flat_out(scrB[b][3:4, :], p2r, 128)
        P_enc = p_slot()
        nc.sync.dma_start(P_enc[0:3, :], scrA[b][0:3, :])
        nc.sync.dma_start(P_enc[3:4, :], ones4096[:, :])

        # conv1 -> conv2 -> x (chunks of 512 points)
        x_sb = px.tile([128, N], F32R, tag="x", name="x")
        gfp = px.tile([128, 8], F32, tag="gfp")
        for ch in range(8):
            ps1 = psum(64, 512)
            nc.tensor.matmul(ps1[:], w1T_sb[:], P_enc[0:3, ts(ch, 512)],
                             start=True, stop=True)
            x1c = px.tile([64, 512], F32R, tag="x1c")
            nc.scalar.activation(x1c[:], ps1[:], AF.Relu, bias=b1_sb[:, 0:1])
            ps2 = psum(128, 512)
            nc.tensor.matmul(ps2[:], w2T_sb[:], x1c[:], start=True, stop=True)
            nc.scalar.activation(x_sb[:, ts(ch, 512)], ps2[:], AF.Relu,
                                 bias=b2_sb[:, 0:1])
        xf = x_sb[:].bitcast(F32)
        # per-chunk maxes then fold (3D reduce in one op)
        nc.vector.tensor_reduce(gfp[:], xf.rearrange("p (a c) -> p a c", c=512),
                                axis=AX.X, op=AL.max)
        gf = px.tile([128, 1], F32, tag="gf")
        nc.vector.tensor_reduce(gf[:], gfp[:], axis=AX.X, op=AL.max)
        # duplicated into 2 cols: f32r matmuls need moving free dim >= 2
        gfr2 = px.tile([128, 2], F32R, tag="gfr2")
        zt2 = px.tile([128, 2], F32, tag="zt2")
        nc.vector.memset(zt2[:], 0.0)
        nc.vector.tensor_scalar(gfr2[:], zt2[:], gf[:, 0:1], None, AL.add)

        # x0 output: lower half = x, upper half = broadcast gfeat
        nc.sync.dma_start(x0_out[b, 0:128, :], xf)
        gfbc = px.tile([128, 512], F32, tag="gfbc")
        zt = px.tile([128, 512], F32, tag="zt")
        nc.vector.memset(zt[:], 0.0)
        nc.vector.tensor_scalar(gfbc[:], zt[:], gf[:, 0:1], None, AL.add)
        for ch in range(8):
            nc.sync.dma_start(x0_out[b, 128:256, ts(ch, 512)], gfbc[:])

        # c3 = W3b @ gfeat + b3 (per-channel constant from the rank-1 half)
        c3_sb = px.tile([128, 8], F32, tag="c3")
        for mt in range(8):
            psc = psum(128, 2)
            nc.tensor.matmul(psc[:], w3bT_sb[:, ts(mt, 128)], gfr2[:],
                             start=True, stop=True)
            nc.scalar.activation(c3_sb[:, mt : mt + 1], psc[:, 0:1], AF.Identity,
                                 bias=b3c_sb[:, mt : mt + 1])

        # conv3 (A half) + channel max + relu -> xmax (f32-exact reduce)
        for mt in range(8):
            gm = pr.tile([128, 2], F32, tag="gm")
            for g in range(2):
                ps3 = psum(128, 2048)
                for j in range(4):
                    nc.tensor.matmul(ps3[:, ts(j, 512)],
                                     w3aT_sb[:, ts(mt, 128)],
                                     x_sb[:, ts(4 * g + j, 512)],
                                     start=True, stop=True)
                nc.vector.tensor_reduce(gm[:, g : g + 1], ps3[:], axis=AX.X,
                                        op=AL.max)
            amax = pr.tile([128, 1], F32, tag="amax")
            nc.vector.tensor_reduce(amax[:], gm[:], axis=AX.X, op=AL.max)
            nc.scalar.activation(xmax_all[:, mt, b : b + 1], amax[:], AF.Relu,
                                 bias=c3_sb[:, mt : mt + 1])

    # ======================= decoder (both items) ==========================
    emb_all = pw.tile([128, 2, 2], F32R)
    for u in range(2):
        pse = psum(128, 2)
        for t in range(8):
            nc.tensor.matmul(pse[:], fencT_sb[t][:, ts(u, 128)],
                             xmax_all[:, t, :], start=(t == 0), stop=(t == 7))
        nc.scalar.activation(emb_all[:, u, :], pse[:], AF.Identity,
                             bias=fbe_sb[:, u : u + 1])
        for b in range(B_L):
            nc.sync.dma_start(
                emb_out[b : b + 1, ts(u, 128)].rearrange("o (p f) -> (o p) f", p=128),
                emb_all[:, u, b : b + 1].bitcast(F32))

    h1_all = pw.tile([128, 4, 2], F32R)
    for mt in range(4):
        ps = psum(128, 2)
        for u in range(2):
            nc.tensor.matmul(ps[:], f1T_sb[u][:, ts(mt, 128)], emb_all[:, u, :],
                             start=(u == 0), stop=(u == 1))
        nc.scalar.activation(h1_all[:, mt, :], ps[:], AF.Relu,
                             bias=fb1_sb[:, mt : mt + 1])

    h2_all = pw.tile([128, 8, 2], F32R)
    for mt in range(8):
        ps = psum(128, 2)
        for u in range(4):
            nc.tensor.matmul(ps[:], f2T_sb[u][:, ts(mt, 128)], h1_all[:, u, :],
                             start=(u == 0), stop=(u == 3))
        nc.scalar.activation(h2_all[:, mt, :], ps[:], AF.Relu,
                             bias=fb2_sb[:, mt : mt + 1])
    h2b = pw.tile([128, 8, 2], BF16)
    nc.vector.tensor_scalar(h2b[:], h2_all[:].bitcast(F32), 0.0, None, AL.add)

    for mt in range(48):
        ps = psum(128, 2)
        for t in range(8):
            nc.tensor.matmul(ps[:], f3T_sb[t][:, ts(mt, 128)], h2b[:, t, :],
                             start=(t == 0), stop=(t == 7))
        nc.scalar.activation(h3_all[:, mt, :], ps[:], AF.Identity,
                             bias=fb3_sb[:, mt : mt + 1])

    # ---- layernorm + out_pc + Q construction per item ---------------------
    for b in range(B_L):
        h3v = h3_all[:, :, b]            # [128, 48] strided view
        s1 = pr.tile([128, 1], F32, tag="s1")
        nc.vector.tensor_reduce(s1[:], h3v, axis=AX.X, op=AL.add)
        sq48 = pr.tile([128, 48], F32, tag="sq48")
        nc.scalar.activation(sq48[:], h3v, AF.Square)
        s2 = pr.tile([128, 1], F32, tag="s2")
        nc.vector.tensor_reduce(s2[:], sq48[:], axis=AX.X, op=AL.add)
        spair = pr.tile([128, 2], F32, tag="spair")
        nc.vector.tensor_copy(spair[:, 0:1], s1[:])
        nc.vector.tensor_copy(spair[:, 1:2], s2[:])
        red = psum(1, 2)
        nc.tensor.matmul(red[:], ones128_sb[:], spair[:], start=True, stop=True)
        mu = pr.tile([1, 1], F32, tag="mu")
        nc.vector.tensor_scalar(mu[:], red[0:1, 0:1], 1.0 / H6, None, AL.mult)
        ms = pr.tile([1, 1], F32, tag="ms")
        nc.vector.tensor_scalar(ms[:], red[0:1, 1:2], 1.0 / H6, None, AL.mult)
        mu2 = pr.tile([1, 1], F32, tag="mu2")
        nc.vector.tensor_mul(mu2[:], mu[:], mu[:])
        var = pr.tile([1, 1], F32, tag="var")
        nc.vector.tensor_sub(var[:], ms[:], mu2[:])
        nc.vector.tensor_scalar_add(var[:], var[:], LN_EPS)
        sd = pr.tile([1, 1], F32, tag="sd")
        nc.scalar.sqrt(sd[:], var[:])
        rstd = pr.tile([1, 1], F32, tag="rstd")
        nc.vector.reciprocal(rstd[:], sd[:])
        nmr = pr.tile([1, 1], F32, tag="nmr")
        nc.vector.tensor_mul(nmr[:], mu[:], rstd[:])
        nc.vector.tensor_scalar(nmr[:], nmr[:], -1.0, None, AL.mult)
        rstd_bc = pr.tile([128, 1], F32, tag="rstd_bc")
        nc.gpsimd.partition_broadcast(rstd_bc[:], rstd[:])
        nmr_bc = pr.tile([128, 1], F32, tag="nmr_bc")
        nc.gpsimd.partition_broadcast(nmr_bc[:], nmr[:])

        h3ln = px.tile([128, 48], F32, tag=f"h3ln{b}")
        nc.scalar.activation(h3ln[:], h3v, AF.Identity,
                             bias=nmr_bc[:, 0:1], scale=rstd_bc[:, 0:1])
        nc.vector.tensor_mul(h3ln[:], h3ln[:], g_sb[:])
        nc.vector.tensor_add(h3ln[:], h3ln[:], bt_sb[:])

        # out_pc (j = 48p + f ordering matches host-side fc3 permutation)
        nc.sync.dma_start(
            pc_out[b : b + 1, :].rearrange("o (p f) -> (o p) f", p=128), h3ln[:])

        # Q planes (out point m = 16p + j)
        h3q = h3ln[:].rearrange("p (a c) -> p a c", c=3)
        qplane = [pr.tile([128, 16], F32R, tag=f"qplane{c}", name=f"qplane{c}") for c in range(3)]
        nqplane = [pr.tile([128, 16], F32R, tag=f"nqplane{c}", name=f"nqplane{c}") for c in range(3)]
        for c in range(3):
            nc.vector.tensor_scalar(qplane[c][:], h3q[:, :, c], 0.0, None, AL.add)
            nc.vector.tensor_scalar(nqplane[c][:], h3q[:, :, c], -2.0, None, AL.mult)
        sqq = pr.tile([128, 48], F32, tag="sqq")
        nc.scalar.activation(sqq[:], h3ln[:], AF.Square)
        sqv = sqq[:].rearrange("p (a c) -> p a c", c=3)
        q2f = pr.tile([128, 16], F32, tag="q2f")
        nc.vector.tensor_add(q2f[:], sqv[:, :, 0], sqv[:, :, 1])
        nc.vector.tensor_add(q2f[:], q2f[:], sqv[:, :, 2])
        q2r = pr.tile([128, 16], F32R, tag="q2r")
        nc.vector.tensor_scalar(q2r[:], q2f[:], 0.0, None, AL.add)
        nc.vector.reduce_sum(stats4[b][:, 3:4], q2f[:], axis=AX.X)

        for c in range(3):
            flat_out(scrQA[b][c : c + 1, :], qplane[c], 128)
            flat_out(scrQB[b][c : c + 1, :], nqplane[c], 128)
        flat_out(scrQB[b][3:4, :], q2r, 128)

    # ======================= chamfer per item ==============================
    for b in range(B_L):
        # d1 direction: per n-tile [128 pts, 2048 out-pts] -> min over m
        P_d1 = p_slot()
        nc.sync.dma_start(P_d1[0:3, :], scrA[b][0:3, :])
        nc.sync.dma_start(P_d1[3:4, :], ones4096[:, :])
        Q_d1 = q_slot()
        nc.sync.dma_start(Q_d1[:, :], scrQB[b][:, :])
        e1min = px.tile([128, 32], F32, tag="e1min")
        for nt in range(16):
            l1g = pr.tile([128, 2, 1024], BF16, tag="l1g")
            for g in range(2):
                ps = psum(128, 2048)
                for j in range(4):
                    nc.tensor.matmul(ps[:, ts(j, 512)], P_d1[:, ts(2 * nt + g, 128)],
                                     Q_d1[:, ts(j, 512)], start=True, stop=True)
                evac = pr.tile([128, 1024], BF16, tag="evac")
                nc.scalar.copy(evac[:], ps[:, 1024:2048])
                nc.vector.tensor_tensor(l1g[:, g, :], ps[:, 0:1024], evac[:],
                                        op=AL.min)
            nc.vector.tensor_reduce(e1min[:, 2 * nt : 2 * nt + 2], l1g[:],
                                    axis=AX.X, op=AL.min)
        nc.vector.reduce_sum(stats4[b][:, 0:1], e1min[:], axis=AX.X)

        # d2 direction: per m-tile [128 out-pts, 4096 pts] -> min over n
        P_d2 = p_slot()
        nc.sync.dma_start(P_d2[:, :], scrB[b][:, :])
        Q_d2 = q_slot()
        nc.sync.dma_start(Q_d2[0:3, :], scrQA[b][0:3, :])
        nc.sync.dma_start(Q_d2[3:4, :], ones4096[:, 0:NPTS])
        e2min = px.tile([128, 16], F32, tag="e2min")
        for mt in range(16):
            l2g = pr.tile([128, 2, 1024], BF16, tag="l1g")
            for g in range(2):
                ps = psum(128, 2048)
                for j in range(4):
                    nc.tensor.matmul(ps[:, ts(j, 512)], Q_d2[:, ts(mt, 128)],
                                     P_d2[:, ts(4 * g + j, 512)],
                                     start=True, stop=True)
                evac = pr.tile([128, 1024], BF16, tag="evac")
                nc.scalar.copy(evac[:], ps[:, 1024:2048])
                nc.vector.tensor_tensor(l2g[:, g, :], ps[:, 0:1024], evac[:],
                                        op=AL.min)
            nc.vector.tensor_reduce(e2min[:, mt : mt + 1], l2g[:],
                                    axis=AX.XY, op=AL.min)
        nc.vector.reduce_sum(stats4[b][:, 2:3], e2min[:], axis=AX.X)

        # loss partial for this item
        red4 = psum(1, 4)
        nc.tensor.matmul(red4[:], ones128_sb[:], stats4[b][:], start=True,
                         stop=True)
        lw = pr.tile([1, 4], F32, tag="lw")
        nc.vector.tensor_mul(lw[:], red4[0:1, :], wloss_sb[:])
        litem = pr.tile([1, 1], F32, tag="litem")
        nc.vector.tensor_reduce(litem[:], lw[:], axis=AX.X, op=AL.add)
        nc.vector.tensor_add(loss_acc[:], loss_acc[:], litem[:])

    nc.sync.dma_start(loss_out[:, :], loss_acc[:])


def build_nc():
    nc = bacc.Bacc("TRN2", target_bir_lowering=False, debug=False)

    def din(name, shape, dt=F32):
        return nc.dram_tensor(name, shape, dt, kind="ExternalInput")

    def dout(name, shape, dt=F32):
        return nc.dram_tensor(name, shape, dt, kind="ExternalOutput")

    tensors = (
        din("pc_raw", [B_L, 128, 96]),
        din("w1T", [3, 64], F32R), din("b1", [64, 1]),
        din("w2T", [64, 128], F32R), din("b2", [128, 1]),
        din("w3aT", [128, 1024], F32R), din("w3bT", [128, 1024], F32R),
        din("b3c", [128, 8]),
        din("fencT", [1024, 256], F32R), din("fbe", [128, 2]),
        din("f1T", [256, 512], F32R), din("fb1", [128, 4]),
        din("f2T", [512, 1024], F32R), din("fb2", [128, 8]),
        din("f3T", [1024, H6], BF16), din("fb3", [128, 48]),
        din("g_ln", [128, 48]), din("bt_ln", [128, 48]),
        din("ones4096", [1, N], F32R),
        din("wloss", [1, 4]),
        dout("x0_out", [B_L, 256, N]),
        dout("emb_out", [B_L, EMB]),
        dout("pc_out", [B_L, H6]),
        dout("loss_out", [1, 1]),
    )
    with tile.TileContext(nc) as tc, ExitStack() as ctx:
        _emit(nc, tc, ctx, tensors)
    nc.compile()
    return nc


# --------------------------------------------------------------------------
# host-side prep / sharding / gather
# --------------------------------------------------------------------------

def _prep_weights(inputs):
    f32 = np.float32
    w = {}
    w["w1T"] = np.ascontiguousarray(inputs["conv1_w"].T.astype(f32))
    w["b1"] = inputs["conv1_b"].astype(f32).reshape(64, 1)
    w["w2T"] = np.ascontiguousarray(inputs["conv2_w"].T.astype(f32))
    w["b2"] = inputs["conv2_b"].astype(f32).reshape(128, 1)
    c3w = inputs["conv3_w"].astype(f32)          # (1024, 256)
    w["w3aT"] = np.ascontiguousarray(c3w[:, 0:128].T)
    w["w3bT"] = np.ascontiguousarray(c3w[:, 128:256].T)
    w["b3c"] = np.ascontiguousarray(inputs["conv3_b"].astype(f32).reshape(8, 128).T)
    w["fencT"] = np.ascontiguousarray(inputs["fc_enc_w"].astype(f32).T)
    w["fbe"] = np.ascontiguousarray(inputs["fc_enc_b"].astype(f32).reshape(2, 128).T)
    w["f1T"] = np.ascontiguousarray(inputs["fc1_w"].astype(f32).T)
    w["fb1"] = np.ascontiguousarray(inputs["fc1_b"].astype(f32).reshape(4, 128).T)
    w["f2T"] = np.ascontiguousarray(inputs["fc2_w"].astype(f32).T)
    w["fb2"] = np.ascontiguousarray(inputs["fc2_b"].astype(f32).reshape(8, 128).T)
    # fc3: permute rows so out partition p of m-tile t holds j = 48p + t
    f3 = inputs["fc3_w"].astype(f32)             # (6144, 1024)
    f3p = f3.reshape(128, 48, 1024).transpose(2, 1, 0).reshape(1024, H6)
    w["f3T"] = np.ascontiguousarray(f3p.astype(ml_dtypes.bfloat16))
    w["fb3"] = np.ascontiguousarray(inputs["fc3_b"].astype(f32).reshape(128, 48))
    w["g_ln"] = np.ascontiguousarray(inputs["ln_g"].astype(f32).reshape(128, 48))
    w["bt_ln"] = np.ascontiguousarray(inputs["ln_b"].astype(f32).reshape(128, 48))
    w["ones4096"] = np.ones((1, N), f32)
    w["wloss"] = np.array([[1.0 / (B * N), 1.0 / (B * N),
                            1.0 / (B * NPTS), 1.0 / (B * NPTS)]], f32)
    return w


def kernel(**inputs):
    if "nc" not in _CACHE:
        _CACHE["nc"] = build_nc()
    nc = _CACHE["nc"]

    w = _prep_weights(inputs)
    in_pc = np.asarray(inputs["in_pc"], np.float32)          # (16, 4096, 3)
    in_maps = []
    for c in range(NCORES):
        m = dict(w)
        m["pc_raw"] = np.ascontiguousarray(
            in_pc[c * B_L : (c + 1) * B_L].reshape(B_L, 128, 96))
        in_maps.append(m)

    res = run_bass_kernel_spmd(nc, in_maps, list(range(NCORES)), trace=False)

    x0 = np.concatenate([r["x0_out"] for r in res.results], axis=0)
    emb = np.concatenate([r["emb_out"] for r in res.results], axis=0)
    out_pc = np.concatenate(
        [r["pc_out"].reshape(B_L, NPTS, 3) for r in res.results], axis=0)
    loss = np.float32(sum(float(r["loss_out"][0, 0]) for r in res.results))
    return (x0, emb, out_pc, loss)


# revision 21
# speedup vs baseline: 1.3010x; 1.3010x over previous
"""PointCloudAE forward pass on 8 Trainium2 NeuronCores (Bass/Tile).

Strategy: data-parallel over batch B=16 -> 2 items per core. Each core runs
encoder (1x1 convs + global max pool), decoder MLP + layernorm, and the
chamfer distance terms for its 2 items; the scalar loss partial is summed on
host across cores (the only cross-core reduction).

Key kernel-level choices:
- All heavy matmuls in float32r (full PE rate, ~1.6e-4 rel err) except fc3
  (bf16 weights, 12 MiB instead of 24 MiB in SBUF).
- x0's upper half (broadcast global feature) is rank-1: conv3's second
  K-tile collapses to a per-channel constant c3 = W3b @ gfeat, halving
  conv3 matmul work.
- Chamfer distance matrix is produced directly by the PE via augmented
  K=4 matmuls: [px,py,pz,1] . [-2qx,-2qy,-2qz,q2] = q2 - 2 p.q. The
  p2[n]/q2[m] terms fall out of the min and are summed separately
  (mean(d1) = mean(min-term) + mean(p2)), so no layout pairing is needed.
- Min/max reductions over PSUM: ACT evacuates half of each strip to SBUF,
  DVE does tensor_tensor-min against the other PSUM half, then reduce_min.
"""
import numpy as np
import ml_dtypes
from contextlib import ExitStack

import concourse.mybir as mybir
import concourse.tile as tile
from concourse import bacc
from concourse.bass_utils import run_bass_kernel_spmd
from concourse.bass import ts

F32 = mybir.dt.float32
F32R = mybir.dt.float32r
BF16 = mybir.dt.bfloat16
AL = mybir.AluOpType
AF = mybir.ActivationFunctionType
AX = mybir.AxisListType

B, N, NPTS = 16, 4096, 2048
B_L = 2                      # items per core
NCORES = 8
EMB = 256
H6 = 3 * NPTS                # 6144
LN_EPS = 1e-5
NEG_INF = -3.0e38
POS_INF = 3.0e38

_CACHE = {}


# --------------------------------------------------------------------------
# program emission
# --------------------------------------------------------------------------

def _emit(nc, tc, ctx, tensors):
    (pc_raw, w1T, b1, w2T, b2, w3aT, w3bT, b3c, fencT, fbe, f1T, fb1,
     f2T, fb2, f3T, fb3, g_ln, bt_ln, ones4096, wloss,
     x0_out, emb_out, pc_out, loss_out) = tensors

    pw = ctx.enter_context(tc.tile_pool(name="weights", bufs=1))
    px = ctx.enter_context(tc.tile_pool(name="work", bufs=1))
    pr = ctx.enter_context(tc.tile_pool(name="red", bufs=2))
    pp = ctx.enter_context(tc.tile_pool(name="ps", bufs=2, space="PSUM"))

    def psum(p_, f_):
        return pp.tile([p_, f_], F32, tag="ps", name="ps")

    # ---- inputs first (so the point cloud isn't queued behind weights) ----
    raws = [px.tile([128, 96], F32, tag=f"raw{b}", name=f"raw{b}") for b in range(B_L)]
    for b in range(B_L):
        nc.sync.dma_start(raws[b][:], pc_raw[b, :, :])

    # ---- persistent weights ----------------------------------------------
    w1T_sb = pw.tile([3, 64], F32R)
    nc.sync.dma_start(w1T_sb[:], w1T[:])
    b1_sb = pw.tile([64, 1], F32)
    nc.sync.dma_start(b1_sb[:], b1[:])
    w2T_sb = pw.tile([64, 128], F32R)
    nc.sync.dma_start(w2T_sb[:], w2T[:])
    b2_sb = pw.tile([128, 1], F32)
    nc.sync.dma_start(b2_sb[:], b2[:])
    w3aT_sb = pw.tile([128, 1024], F32R)
    nc.gpsimd.dma_start(w3aT_sb[:], w3aT[:])
    w3bT_sb = pw.tile([128, 1024], F32R)
    nc.gpsimd.dma_start(w3bT_sb[:], w3bT[:])
    b3c_sb = pw.tile([128, 8], F32)
    nc.sync.dma_start(b3c_sb[:], b3c[:])
    fbe_sb = pw.tile([128, 2], F32)
    nc.sync.dma_start(fbe_sb[:], fbe[:])
    f1T_sb = [pw.tile([128, 512], F32R, tag=f"f1T{t}", name=f"f1T{t}") for t in range(2)]
    for t in range(2):
        nc.gpsimd.dma_start(f1T_sb[t][:], f1T[ts(t, 128), :])
    fb1_sb = pw.tile([128, 4], F32)
    nc.sync.dma_start(fb1_sb[:], fb1[:])
    fb2_sb = pw.tile([128, 8], F32)
    nc.sync.dma_start(fb2_sb[:], fb2[:])
    f3T_sb = [pw.tile([128, H6], BF16, tag=f"f3T{t}", name=f"f3T{t}") for t in range(8)]
    fencT_sb = [pw.tile([128, 256], F32R, tag=f"fencT{t}", name=f"fencT{t}") for t in range(8)]
    f2T_sb = [pw.tile([128, 1024], F32R, tag=f"f2T{t}", name=f"f2T{t}") for t in range(4)]
    fb3_sb = pw.tile([128, 48], F32)
    nc.sync.dma_start(fb3_sb[:], fb3[:])
    g_sb = pw.tile([128, 48], F32)
    nc.sync.dma_start(g_sb[:], g_ln[:])
    bt_sb = pw.tile([128, 48], F32)
    nc.sync.dma_start(bt_sb[:], bt_ln[:])
    ones128_sb = pw.tile([128, 1], F32)
    nc.vector.memset(ones128_sb[:], 1.0)
    wloss_sb = pw.tile([1, 4], F32)
    nc.sync.dma_start(wloss_sb[:], wloss[:])

    xmax_all = pw.tile([128, 8, 2], F32R)
    h3_all = pw.tile([128, 48, 2], F32)
    loss_acc = pw.tile([1, 1], F32)
    nc.vector.memset(loss_acc[:], 0.0)
    stats4 = [pw.tile([128, 4], F32, tag=f"stats{b}", name=f"stats{b}") for b in range(B_L)]

    scrA = [nc.dram_tensor(f"scrA{b}", [4, N], F32R) for b in range(B_L)]
    scrB = [nc.dram_tensor(f"scrB{b}", [4, N], F32R) for b in range(B_L)]
    scrQA = [nc.dram_tensor(f"scrQA{b}", [4, NPTS], F32R) for b in range(B_L)]
    scrQB = [nc.dram_tensor(f"scrQB{b}", [4, NPTS], F32R) for b in range(B_L)]

    def flat_out(dram_row, sb_tile):
        nc.sync.dma_start(
            dram_row.rearrange("o (p f) -> (o p) f", p=128), sb_tile[:])

    def p_slot():
        return px.tile([4, N], F32R, tag="Pslot", name="Pslot", bufs=1)

    def q_slot():
        return px.tile([4, NPTS], F32R, tag="Qslot", name="Qslot", bufs=1)

    # strip reduction: [128, 2048] PSUM -> 1024 bf16 partial min/max in l1g
    def strip_fold(ps, dst, op):
        evac = pr.tile([128, 1536], BF16, tag="evac", name="evac")
        nc.scalar.copy(evac[:], ps[:, 512:2048])
        nc.vector.tensor_tensor(dst[:, 0:512], ps[:, 0:512], evac[:, 0:512], op=op)
        nc.vector.tensor_tensor(dst[:, 512:1024], evac[:, 512:1024],
                                evac[:, 1024:1536], op=op)

    def fold_tail(l1g, out_ap, axis, op):
        f2 = pr.tile([128, 2, 512], BF16, tag="f2", name="f2")
        nc.vector.tensor_tensor(f2[:], l1g[:, :, 0:512], l1g[:, :, 512:1024], op=op)
        f3 = pr.tile([128, 2, 256], BF16, tag="f3", name="f3")
        nc.vector.tensor_tensor(f3[:], f2[:, :, 0:256], f2[:, :, 256:512], op=op)
        nc.vector.tensor_reduce(out_ap, f3[:], axis=axis, op=op)

    # ======================= per-item encoder ==============================
    for b in range(B_L):
        raw = raws[b]
        rv = raw[:].rearrange("p (a c) -> p a c", c=3)
        plane = [px.tile([128, 32], F32R, tag=f"plane{c}", name=f"plane{c}") for c in range(3)]
        nplane = [px.tile([128, 32], F32R, tag=f"nplane{c}", name=f"nplane{c}") for c in range(3)]
        for c in range(3):
            nc.vector.tensor_scalar(plane[c][:], rv[:, :, c], 0.0, None, AL.add)
            nc.vector.tensor_scalar(nplane[c][:], rv[:, :, c], -2.0, None, AL.mult)
        sq = px.tile([128, 96], F32, tag="sq")
        nc.scalar.activation(sq[:], raw[:], AF.Square)
        sv = sq[:].rearrange("p (a c) -> p a c", c=3)
        t2 = px.tile([128, 32], F32, tag="t2")
        nc.vector.tensor_add(t2[:], sv[:, :, 0], sv[:, :, 1])
        nc.vector.tensor_add(t2[:], t2[:], sv[:, :, 2])
        p2r = px.tile([128, 32], F32R, tag="p2r")
        nc.vector.tensor_scalar(p2r[:], t2[:], 0.0, None, AL.add)
        nc.vector.reduce_sum(stats4[b][:, 1:2], t2[:], axis=AX.X)

        for c in range(3):
            flat_out(scrA[b][c : c + 1, :], plane[c])
            flat_out(scrB[b][c : c + 1, :], nplane[c])
        flat_out(scrB[b][3:4, :], p2r)
        P_A = p_slot()
        nc.sync.dma_start(P_A[0:3, :], scrA[b][0:3, :])
        nc.sync.dma_start(P_A[3:4, :], ones4096[:, :])

        x_sb = px.tile([128, N], F32R, tag="x", name="x")
        gfp = px.tile([128, 8], F32, tag="gfp")
        for ch in range(8):
            ps1 = psum(64, 512)
            nc.tensor.matmul(ps1[:], w1T_sb[:], P_A[0:3, ts(ch, 512)],
                             start=True, stop=True)
            x1c = px.tile([64, 512], F32R, tag="x1c")
            nc.scalar.activation(x1c[:], ps1[:], AF.Relu, bias=b1_sb[:, 0:1])
            ps2 = psum(128, 512)
            nc.tensor.matmul(ps2[:], w2T_sb[:], x1c[:], start=True, stop=True)
            nc.scalar.activation(x_sb[:, ts(ch, 512)], ps2[:], AF.Relu,
                                 bias=b2_sb[:, 0:1])
        xf = x_sb[:].bitcast(F32)
        nc.vector.tensor_reduce(gfp[:], xf.rearrange("p (a c) -> p a c", c=512),
                                axis=AX.X, op=AL.max)
        gf = px.tile([128, 1], F32, tag="gf")
        nc.vector.tensor_reduce(gf[:], gfp[:], axis=AX.X, op=AL.max)
        gfr2 = px.tile([128, 2], F32R, tag="gfr2")
        nc.scalar.activation(gfr2[:], fbe_sb[:], AF.Identity,
                             bias=gf[:, 0:1], scale=0.0)

        nc.sync.dma_start(x0_out[b, 0:128, :], xf)
        gfbc = px.tile([128, 256], F32, tag="gfbc")
        nc.scalar.activation(gfbc[:], x_sb[:, 0:256].bitcast(F32), AF.Identity,
                             bias=gf[:, 0:1], scale=0.0)
        for ch in range(16):
            nc.sync.dma_start(x0_out[b, 128:256, ts(ch, 256)], gfbc[:])

        c3_sb = px.tile([128, 8], F32, tag="c3")
        for mt in range(8):
            psc = psum(128, 2)
            nc.tensor.matmul(psc[:], w3bT_sb[:, ts(mt, 128)], gfr2[:],
                             start=True, stop=True)
            nc.scalar.activation(c3_sb[:, mt : mt + 1], psc[:, 0:1], AF.Identity,
                                 bias=b3c_sb[:, mt : mt + 1])

        for mt in range(8):
            l1g = pr.tile([128, 2, 1024], BF16, tag="l1g")
            for g in range(2):
                ps3 = psum(128, 2048)
                for j in range(4):
                    nc.tensor.matmul(ps3[:, ts(j, 512)],
                                     w3aT_sb[:, ts(mt, 128)],
                                     x_sb[:, ts(4 * g + j, 512)],
                                     start=True, stop=True)
                strip_fold(ps3, l1g[:, g, :], AL.max)
            amax = pr.tile([128, 1], F32, tag="amax")
            fold_tail(l1g, amax[:], AX.XY, AL.max)
            nc.scalar.activation(xmax_all[:, mt, b : b + 1], amax[:], AF.Relu,
                                 bias=c3_sb[:, mt : mt + 1])

        if b == 0:
            # big decoder weights stream in while item 1's encoder runs
            for t in range(8):
                nc.gpsimd.dma_start(fencT_sb[t][:], fencT[ts(t, 128), :])
            for t in range(2):
                pass
            for t in range(4):
                nc.gpsimd.dma_start(f2T_sb[t][:], f2T[ts(t, 128), :])
            for t in range(8):
                nc.gpsimd.dma_start(f3T_sb[t][:], f3T[ts(t, 128), :])

    # ======================= decoder (both items) ==========================
    emb_all = pw.tile([128, 2, 2], F32R)
    for u in range(2):
        pse = psum(128, 2)
        for t in range(8):
            nc.tensor.matmul(pse[:], fencT_sb[t][:, ts(u, 128)],
                             xmax_all[:, t, :], start=(t == 0), stop=(t == 7))
        nc.scalar.activation(emb_all[:, u, :], pse[:], AF.Identity,
                             bias=fbe_sb[:, u : u + 1])
        for b in range(B_L):
            nc.sync.dma_start(
                emb_out[b : b + 1, ts(u, 128)].rearrange("o (p f) -> (o p) f", p=128),
                emb_all[:, u, b : b + 1].bitcast(F32))

    h1_all = pw.tile([128, 4, 2], F32R)
    for mt in range(4):
        ps = psum(128, 2)
        for u in range(2):
            nc.tensor.matmul(ps[:], f1T_sb[u][:, ts(mt, 128)], emb_all[:, u, :],
                             start=(u == 0), stop=(u == 1))
        nc.scalar.activation(h1_all[:, mt, :], ps[:], AF.Relu,
                             bias=fb1_sb[:, mt : mt + 1])

    h2b = pw.tile([128, 8, 2], BF16)
    for mt in range(8):
        ps = psum(128, 2)
        for u in range(4):
            nc.tensor.matmul(ps[:], f2T_sb[u][:, ts(mt, 128)], h1_all[:, u, :],
                             start=(u == 0), stop=(u == 3))
        h22c = pr.tile([128, 2], F32R, tag="h22c", name="h22c")
        nc.scalar.activation(h22c[:], ps[:], AF.Relu,
                             bias=fb2_sb[:, mt : mt + 1])
        nc.vector.tensor_scalar(h2b[:, mt, :], h22c[:].bitcast(F32),
                                0.0, None, AL.add)

    for mt in range(48):
        ps = psum(128, 2)
        for t in range(8):
            nc.tensor.matmul(ps[:], f3T_sb[t][:, ts(mt, 128)], h2b[:, t, :],
                             start=(t == 0), stop=(t == 7))
        nc.scalar.activation(h3_all[:, mt, :], ps[:], AF.Identity,
                             bias=fb3_sb[:, mt : mt + 1])

    # ---- layernorm + out_pc + Q construction per item ---------------------
    for b in range(B_L):
        h3v = h3_all[:, :, b]
        s1 = pr.tile([128, 1], F32, tag="s1")
        nc.vector.tensor_reduce(s1[:], h3v, axis=AX.X, op=AL.add)
        sq48 = pr.tile([128, 48], F32, tag="sq48")
        nc.scalar.activation(sq48[:], h3v, AF.Square)
        s2 = pr.tile([128, 1], F32, tag="s2")
        nc.vector.tensor_reduce(s2[:], sq48[:], axis=AX.X, op=AL.add)
        spair = pr.tile([128, 2], F32, tag="spair")
        nc.vector.tensor_copy(spair[:, 0:1], s1[:])
        nc.vector.tensor_copy(spair[:, 1:2], s2[:])
        red = psum(1, 2)
        nc.tensor.matmul(red[:], ones128_sb[:], spair[:], start=True, stop=True)
        mu = pr.tile([1, 1], F32, tag="mu")
        nc.vector.tensor_scalar(mu[:], red[0:1, 0:1], 1.0 / H6, None, AL.mult)
        ms = pr.tile([1, 1], F32, tag="ms")
        nc.vector.tensor_scalar(ms[:], red[0:1, 1:2], 1.0 / H6, None, AL.mult)
        mu2 = pr.tile([1, 1], F32, tag="mu2")
        nc.vector.tensor_mul(mu2[:], mu[:], mu[:])
        var = pr.tile([1, 1], F32, tag="var")
        nc.vector.tensor_sub(var[:], ms[:], mu2[:])
        nc.vector.tensor_scalar_add(var[:], var[:], LN_EPS)
        sd = pr.tile([1, 1], F32, tag="sd")
        nc.scalar.sqrt(sd[:], var[:])
        rstd = pr.tile([1, 1], F32, tag="rstd")
        nc.vector.reciprocal(rstd[:], sd[:])
        nmr = pr.tile([1, 1], F32, tag="nmr")
        nc.vector.tensor_mul(nmr[:], mu[:], rstd[:])
        nc.vector.tensor_scalar(nmr[:], nmr[:], -1.0, None, AL.mult)
        rstd_bc = pr.tile([128, 1], F32, tag="rstd_bc")
        nc.gpsimd.partition_broadcast(rstd_bc[:], rstd[:])
        nmr_bc = pr.tile([128, 1], F32, tag="nmr_bc")
        nc.gpsimd.partition_broadcast(nmr_bc[:], nmr[:])

        h3ln = px.tile([128, 48], F32, tag="h3ln", name="h3ln", bufs=2)
        nc.scalar.activation(h3ln[:], h3v, AF.Identity,
                             bias=nmr_bc[:, 0:1], scale=rstd_bc[:, 0:1])
        nc.vector.tensor_mul(h3ln[:], h3ln[:], g_sb[:])
        nc.vector.tensor_add(h3ln[:], h3ln[:], bt_sb[:])

        nc.sync.dma_start(
            pc_out[b : b + 1, :].rearrange("o (p f) -> (o p) f", p=128), h3ln[:])

        h3q = h3ln[:].rearrange("p (a c) -> p a c", c=3)
        qplane = [pr.tile([128, 16], F32R, tag=f"qplane{c}", name=f"qplane{c}") for c in range(3)]
        nqplane = [pr.tile([128, 16], F32R, tag=f"nqplane{c}", name=f"nqplane{c}") for c in range(3)]
        for c in range(3):
            nc.vector.tensor_scalar(qplane[c][:], h3q[:, :, c], 0.0, None, AL.add)
            nc.vector.tensor_scalar(nqplane[c][:], h3q[:, :, c], -2.0, None, AL.mult)
        sqq = pr.tile([128, 48], F32, tag="sqq")
        nc.scalar.activation(sqq[:], h3ln[:], AF.Square)
        sqv = sqq[:].rearrange("p (a c) -> p a c", c=3)
        q2f = pr.tile([128, 16], F32, tag="q2f")
        nc.vector.tensor_add(q2f[:], sqv[:, :, 0], sqv[:, :, 1])
        nc.vector.tensor_add(q2f[:], q2f[:], sqv[:, :, 2])
        q2r = pr.tile([128, 16], F32R, tag="q2r")
        nc.vector.tensor_scalar(q2r[:], q2f[:], 0.0, None, AL.add)
        nc.vector.reduce_sum(stats4[b][:, 3:4], q2f[:], axis=AX.X)

        for c in range(3):
            flat_out(scrQA[b][c : c + 1, :], qplane[c])
            flat_out(scrQB[b][c : c + 1, :], nqplane[c])
        flat_out(scrQB[b][3:4, :], q2r)

    # ======================= chamfer per item ==============================
    for b in range(B_L):
        P_A = p_slot()
        nc.sync.dma_start(P_A[0:3, :], scrA[b][0:3, :])
        nc.sync.dma_start(P_A[3:4, :], ones4096[:, :])
        Q_B = q_slot()
        nc.sync.dma_start(Q_B[:, :], scrQB[b][:, :])
        e1min = px.tile([128, 32], F32, tag="e1min")
        for nt in range(16):
            l1g = pr.tile([128, 2, 1024], BF16, tag="l1g")
            for g in range(2):
                ps = psum(128, 2048)
                for j in range(4):
                    nc.tensor.matmul(ps[:, ts(j, 512)], P_A[:, ts(2 * nt + g, 128)],
                                     Q_B[:, ts(j, 512)], start=True, stop=True)
                strip_fold(ps, l1g[:, g, :], AL.min)
            fold_tail(l1g, e1min[:, 2 * nt : 2 * nt + 2], AX.X, AL.min)
        nc.vector.reduce_sum(stats4[b][:, 0:1], e1min[:], axis=AX.X)

        P_B = p_slot()
        nc.sync.dma_start(P_B[:, :], scrB[b][:, :])
        Q_A = q_slot()
        nc.sync.dma_start(Q_A[0:3, :], scrQA[b][0:3, :])
        nc.sync.dma_start(Q_A[3:4, :], ones4096[:, 0:NPTS])
        e2min = px.tile([128, 16], F32, tag="e2min")
        for mt in range(16):
            l2g = pr.tile([128, 2, 1024], BF16, tag="l1g", name="l2g")
            for g in range(2):
                ps = psum(128, 2048)
                for j in range(4):
                    nc.tensor.matmul(ps[:, ts(j, 512)], Q_A[:, ts(mt, 128)],
                                     P_B[:, ts(4 * g + j, 512)],
                                     start=True, stop=True)
                strip_fold(ps, l2g[:, g, :], AL.min)
            fold_tail(l2g, e2min[:, mt : mt + 1], AX.XY, AL.min)
        nc.vector.reduce_sum(stats4[b][:, 2:3], e2min[:], axis=AX.X)

        red4 = psum(1, 4)
        nc.tensor.matmul(red4[:], ones128_sb[:], stats4[b][:], start=True,
                         stop=True)
        lw = pr.tile([1, 4], F32, tag="lw")
        nc.vector.tensor_mul(lw[:], red4[0:1, :], wloss_sb[:])
        litem = pr.tile([1, 1], F32, tag="litem")
        nc.vector.tensor_reduce(litem[:], lw[:], axis=AX.X, op=AL.add)
        nc.vector.tensor_add(loss_acc[:], loss_acc[:], litem[:])

    nc.sync.dma_start(loss_out[:, :], loss_acc[:])


def build_nc():
    nc = bacc.Bacc("TRN2", target_bir_lowering=False, debug=False)

    def din(name, shape, dt=F32):
        return nc.dram_tensor(name, shape, dt, kind="ExternalInput")

    def dout(name, shape, dt=F32):
        return nc.dram_tensor(name, shape, dt, kind="ExternalOutput")

    tensors = (
        din("pc_raw", [B_L, 128, 96]),
        din("w1T", [3, 64], F32R), din("b1", [64, 1]),
        din("w2T", [64, 128], F32R), din("b2", [128, 1]),
        din("w3aT", [128, 1024], F32R), din("w3bT", [128, 1024], F32R),
        din("b3c", [128, 8]),
        din("fencT", [1024, 256], F32R), din("fbe", [128, 2]),
        din("f1T", [256, 512], F32R), din("fb1", [128, 4]),
        din("f2T", [512, 1024], F32R), din("fb2", [128, 8]),
        din("f3T", [1024, H6], BF16), din("fb3", [128, 48]),
        din("g_ln", [128, 48]), din("bt_ln", [128, 48]),
        din("ones4096", [1, N], F32R),
        din("wloss", [1, 4]),
        dout("x0_out", [B_L, 256, N]),
        dout("emb_out", [B_L, EMB]),
        dout("pc_out", [B_L, H6]),
        dout("loss_out", [1, 1]),
    )
    with tile.TileContext(nc) as tc, ExitStack() as ctx:
        _emit(nc, tc, ctx, tensors)
    nc.compile()
    return nc


# --------------------------------------------------------------------------
# host-side prep / sharding / gather
# --------------------------------------------------------------------------

def _prep_weights(inputs):
    f32 = np.float32
    w = {}
    w["w1T"] = np.ascontiguousarray(inputs["conv1_w"].T.astype(f32))
    w["b1"] = inputs["conv1_b"].astype(f32).reshape(64, 1)
    w["w2T"] = np.ascontiguousarray(inputs["conv2_w"].T.astype(f32))
    w["b2"] = inputs["conv2_b"].astype(f32).reshape(128, 1)
    c3w = inputs["conv3_w"].astype(f32)          # (1024, 256)
    w["w3aT"] = np.ascontiguousarray(c3w[:, 0:128].T)
    w["w3bT"] = np.ascontiguousarray(c3w[:, 128:256].T)
    w["b3c"] = np.ascontiguousarray(inputs["conv3_b"].astype(f32).reshape(8, 128).T)
    w["fencT"] = np.ascontiguousarray(inputs["fc_enc_w"].astype(f32).T)
    w["fbe"] = np.ascontiguousarray(inputs["fc_enc_b"].astype(f32).reshape(2, 128).T)
    w["f1T"] = np.ascontiguousarray(inputs["fc1_w"].astype(f32).T)
    w["fb1"] = np.ascontiguousarray(inputs["fc1_b"].astype(f32).reshape(4, 128).T)
    w["f2T"] = np.ascontiguousarray(inputs["fc2_w"].astype(f32).T)
    w["fb2"] = np.ascontiguousarray(inputs["fc2_b"].astype(f32).reshape(8, 128).T)
    # fc3: permute rows so out partition p of m-tile t holds j = 48p + t
    f3 = inputs["fc3_w"].astype(f32)             # (6144, 1024)
    f3p = f3.reshape(128, 48, 1024).transpose(2, 1, 0).reshape(1024, H6)
    w["f3T"] = np.ascontiguousarray(f3p.astype(ml_dtypes.bfloat16))
    w["fb3"] = np.ascontiguousarray(inputs["fc3_b"].astype(f32).reshape(128, 48))
    w["g_ln"] = np.ascontiguousarray(inputs["ln_g"].astype(f32).reshape(128, 48))
    w["bt_ln"] = np.ascontiguousarray(inputs["ln_b"].astype(f32).reshape(128, 48))
    w["ones4096"] = np.ones((1, N), f32)
    w["wloss"] = np.array([[1.0 / (B * N), 1.0 / (B * N),
                            1.0 / (B * NPTS), 1.0 / (B * NPTS)]], f32)
    return w


def kernel(**inputs):
    if "nc" not in _CACHE:
        _CACHE["nc"] = build_nc()
    nc = _CACHE["nc"]

    w = _prep_weights(inputs)
    in_pc = np.asarray(inputs["in_pc"], np.float32)          # (16, 4096, 3)
    in_maps = []
    for c in range(NCORES):
        m = dict(w)
        m["pc_raw"] = np.ascontiguousarray(
            in_pc[c * B_L : (c + 1) * B_L].reshape(B_L, 128, 96))
        in_maps.append(m)

    res = run_bass_kernel_spmd(nc, in_maps, list(range(NCORES)), trace=False)

    x0 = np.concatenate([r["x0_out"] for r in res.results], axis=0)
    emb = np.concatenate([r["emb_out"] for r in res.results], axis=0)
    out_pc = np.concatenate(
        [r["pc_out"].reshape(B_L, NPTS, 3) for r in res.results], axis=0)
    loss = np.float32(sum(float(r["loss_out"][0, 0]) for r in res.results))
    return (x0, emb, out_pc, loss)
